# revision 1
# baseline (speedup 1.0000x reference)
"""Trainium2 Bass kernel for nn_Aggregator (gnn_message_passing).

pooled[B,D] = owner_masks.f32 @ ((nodes@Wt + bt) * sigmoid(nodes@Wg + bg))

Sharding: nodes (and owner_masks columns) split along N across 8 cores.
Each core computes a partial [B, 2D] = [M@(A*G) | M@G]; the host sums the
8 partials and applies the bt column bias algebraically:
    pooled = sum_c pool1_c + (sum_c pool2_c) * bt[None, :]
(exact: (A + 1 bt^T) * G = A*G + (1 bt^T)*G and M @ ((1 bt^T)*G) =
(M@G) diag(bt)).

Device pipeline (per core; fp16 inputs, fp32 accumulation). DMA loads come
in multi-chunk slabs (few, large HWDGE dispatches); compute runs per chunk
of 8 128-node tiles:
    PE : MM(psum_d[:,sl], lhsT=nodesT[:,sl], rhs=Wt)  (A tile, [n,D])
         MM(psum_g[:,sl], lhsT=nodesT[:,sl], rhs=Wg)  (G_pre,  [n,D])
    DVE: gpre = psum_g + bg_bcast          (fp16 out)
    ACT: mg[:, :, :D]  = copy(psum_d)      (fp16)
    ACT: mg[:, :, D:]  = sigmoid(gpre)     (fp16)
    DVE: mg[:, :, :D] *= mg[:, :, D:]      (in-place msg = A*G)
    PE : pool12[B, 2D] += masksT_t.T @ mg_t   (one 256-wide MM per tile)
Cost-model (TimelineSim) exec time: ~141.6 us/core; engine busy: ACT ~129 us
(sigmoid + psum_d eviction — the bottleneck), DVE ~111 us (bias-add + mul),
PE ~106 us (MAC floor for this algorithm), DMA data ~92 us (fp16 floor),
HWDGE ~24 us. Startup is trimmed by emitting the first node slab right
after wt on the DMA ring and burning the PE HAM clock ramp with 6 warm-up
matmuls during the initial DMA wait. Structure notes from iteration:
keeping copy_d wholly on ACT beats any ACT/DVE alternation or
DVE-psum-fused multiply (those extend psum_d slot lifetime and stall the
3-slot PSUM rotation); GPSIMD tensor ops (0.42 efficiency) rate-limit the
chunk pipeline if placed on the msg path; mask DMAs must stay on the SP
HWDGE ring.
"""

import json

import numpy as np

import concourse.bass as bass
import concourse.mybir as mybir
import concourse.tile as tile
from concourse import bass2jax as _b2j
from concourse import bass_utils as _bu
from concourse.bass_utils import run_bass_kernel_spmd


def _split_excess_waits_json(bir_json) -> bytes:
    """Walrus in this container accepts at most 1 embedded sem-wait per
    instruction (2 for EventSemaphore). Tile emits instructions (notably the
    kernel-tail Drain) with more. Move excess waits onto injected
    EventSemaphore instructions placed immediately before the offender in
    the same engine stream — identical blocking semantics."""
    if isinstance(bir_json, str):
        bir_json = bir_json.encode()
    d = json.loads(bir_json)
    counter = [0]

    def fix_block(b):
        new = []
        for inst in b.get("instructions", []):
            si = inst.get("sync_info")
            waits = (si or {}).get("on_wait") or []
            cap = 2 if inst.get("opcode") == "EventSemaphore" else 1
            if len(waits) > cap:
                keep, excess = waits[:cap], waits[cap:]
                for j in range(0, len(excess), 2):
                    counter[0] += 1
                    new.append(
                        {
                            "debug": inst.get("debug"),
                            "engine": inst["engine"],
                            "ins": [],
                            "outs": [],
                            "name": f"antsplit_ev_{counter[0]}",
                            "opcode": "EventSemaphore",
                            "sync_info": {
                                "on_update": [],
                                "on_wait": excess[j : j + 2],
                            },
                        }
                    )
                si["on_wait"] = keep
            new.append(inst)
        b["instructions"] = new
        for sb in b.get("blocks", []):
            fix_block(sb)

    for f in d.get("functions", []):
        for blk in f.get("blocks", []):
            fix_block(blk)
    return json.dumps(d).encode()


if not getattr(_bu, "_ant_split_waits_patched", False):
    _orig_compile_bir_kernel = _bu.compile_bir_kernel

    def _patched_compile_bir_kernel(bir_json, tmpdir, neff_name="file.neff"):
        return _orig_compile_bir_kernel(
            _split_excess_waits_json(bir_json), tmpdir, neff_name
        )

    _bu.compile_bir_kernel = _patched_compile_bir_kernel
    _b2j.compile_bir_kernel = _patched_compile_bir_kernel
    _bu._ant_split_waits_patched = True

N_CORES = 8
N_TOTAL = 500_000
B = 128
S = 128
D = 128
P = 128

N_PER_CORE = N_TOTAL // N_CORES          # 62500
TILES_PER_CHUNK = 8
CHUNK = TILES_PER_CHUNK * P              # 1024
# small first slabs so compute starts early; 62 chunks = 63488 nodes total
SLAB_CHUNKS = [2, 2] + [4] * 14 + [2]
N_CHUNKS = sum(SLAB_CHUNKS)              # 62
N_TILES = N_CHUNKS * TILES_PER_CHUNK     # 496
N_PAD = N_TILES * P                      # 63488

F16 = mybir.dt.float16
F32 = mybir.dt.float32
NP_F16 = np.float16


# bisect/debug switches
OPTS = {
    "touches": True,     # one-time const touch ops
    "sigmoid": True,     # False -> plain Copy instead of Sigmoid
    "mm2": True,         # False -> skip the pooling matmuls (evict psum_d instead)
    "mm1": True,         # False -> skip the feature matmuls entirely
    "elemwise": True,    # False -> skip DVE/ACT elementwise ops
    # chunks whose sigma/mul run as one wide SBUF op. 1 is optimal: larger
    # groups amortize ACT per-op init but the sigma lump stalls the 3-slot
    # PSUM rotation (8-bank budget) and regresses end-to-end.
    "group": 1,
    # trailing tiles of each chunk's psum_d eviction done on DVE instead of
    # ACT. 0 is optimal: any DVE share extends the TT->copy->mul chain and
    # psum_d's lifetime, stalling the PSUM rotation (same failure mode as
    # every other copy_d offload variant).
    "dve_copy_tiles": 0,
    # dummy PE matmuls at kernel start to burn the HAM clock ramp while the
    # first node slab is still in flight
    "warm_mms": 6,
    # emit the first node-slab DMA right after wt (before wg/bgb consts)
    "early_nod0": True,
    # process the first/last chunk in two halves to shorten the serial
    # dependency chain at the pipeline edges. False is optimal: the extra
    # per-op access-latency inits outweigh the halved edge chain.
    "edge_halves": False,
}


def build_bass() -> bass.Bass:
    nc = bass.Bass()

    nodesT = nc.dram_tensor("nodesT", [P, N_PAD], F16, kind="ExternalInput").ap()
    masksT = nc.dram_tensor("masksT", [P, N_TILES, B], F16, kind="ExternalInput").ap()
    wt_d = nc.dram_tensor("wt", [S, D], F16, kind="ExternalInput").ap()
    wg_d = nc.dram_tensor("wg", [S, D], F16, kind="ExternalInput").ap()
    bgb_d = nc.dram_tensor("bgb", [P, CHUNK], F32, kind="ExternalInput").ap()
    out_d = nc.dram_tensor("out", [B, 2 * D], F32, kind="ExternalOutput").ap()

    with tile.TileContext(nc) as tc:
        with (
            tc.tile_pool(name="consts", bufs=1) as consts,
            tc.tile_pool(name="scratch", bufs=1) as scratch,
            tc.tile_pool(name="nodes", bufs=4) as nodes_pool,
            tc.tile_pool(name="masks", bufs=4) as masks_pool,
            tc.tile_pool(name="gpre", bufs=3) as gpre_pool,
            tc.tile_pool(name="dt", bufs=3) as d_pool,
            tc.tile_pool(name="gt", bufs=3) as g_pool,
            tc.tile_pool(name="outs", bufs=1) as out_pool,
            tc.tile_pool(name="ps", bufs=3, space="PSUM") as ps_pool,
            tc.tile_pool(name="acc", bufs=1, space="PSUM") as acc_pool,
        ):
            def emit_slab(s_chunks, slab_off):
                slab_n = s_chunks * CHUNK
                nod_slab = nodes_pool.tile([P, 4 * CHUNK], F16, tag="nod_slab")
                nc.sync.dma_start(
                    nod_slab[:, :slab_n],
                    nodesT[:, slab_off : slab_off + slab_n],
                )
                mk_slab = masks_pool.tile(
                    [P, 4 * TILES_PER_CHUNK, B], F16, tag="mk_slab"
                )
                to = slab_off // P
                nc.sync.dma_start(
                    mk_slab[:, : s_chunks * TILES_PER_CHUNK, :],
                    masksT[:, to : to + s_chunks * TILES_PER_CHUNK, :],
                )
                return nod_slab, mk_slab

            wt_sb = consts.tile([S, D], F16)
            nc.sync.dma_start(wt_sb[:], wt_d)
            if OPTS["early_nod0"]:
                # put the first node slab on the DMA ring right after wt so
                # its (large) transfer overlaps the remaining const loads
                slab_n0 = SLAB_CHUNKS[0] * CHUNK
                nod_slab0 = nodes_pool.tile([P, 4 * CHUNK], F16, tag="nod_slab")
                nc.sync.dma_start(nod_slab0[:, :slab_n0], nodesT[:, :slab_n0])
            wg_sb = consts.tile([S, D], F16)
            nc.sync.dma_start(wg_sb[:], wg_d)
            bgb_sb = consts.tile([P, CHUNK], F32)
            nc.sync.dma_start(bgb_sb[:], bgb_d)

            # One-time const touches: absorb the const-DMA semaphores into
            # each engine's observed clock so hot-loop instructions never
            # need a second (DMA) wait slot.
            if OPTS["touches"]:
                dve_scratch = scratch.tile([1, 2], F32)
                nc.vector.tensor_copy(out=dve_scratch[:1, :1], in_=bgb_sb[:1, :1])
                nc.tensor.ldweights(wt_sb[:, :1])
                nc.tensor.ldweights(wg_sb[:, :1])
            if OPTS["warm_mms"]:
                # burn the PE HAM clock ramp during the initial DMA wait;
                # scratch lives in the rotating psum pool (one-time slot use)
                warm_ps = ps_pool.tile([P, CHUNK], F32, tag="ps")
                for _ in range(OPTS["warm_mms"]):
                    nc.tensor.matmul(
                        warm_ps[:, :D], wt_sb[:], wg_sb[:], start=True, stop=True
                    )
                nc.vector.tensor_copy(
                    out=dve_scratch[:1, 1:2], in_=warm_ps[:1, :1]
                )
            if OPTS["early_nod0"]:
                mk_slab0 = masks_pool.tile(
                    [P, 4 * TILES_PER_CHUNK, B], F16, tag="mk_slab"
                )
                nc.sync.dma_start(
                    mk_slab0[:, : SLAB_CHUNKS[0] * TILES_PER_CHUNK, :],
                    masksT[:, : SLAB_CHUNKS[0] * TILES_PER_CHUNK, :],
                )
                slab0 = (nod_slab0, mk_slab0)
            else:
                slab0 = emit_slab(SLAB_CHUNKS[0], 0)

            if OPTS["mm2"]:
                # pool12[:, :D] accumulates M@(A*G); [:, D:] accumulates M@G
                pool12 = acc_pool.tile([B, 2 * D], F32)

            c = 0
            slab_off = 0
            for s, s_chunks in enumerate(SLAB_CHUNKS):
                if s == 0:
                    nod_slab, mk_slab = slab0
                else:
                    nod_slab, mk_slab = emit_slab(s_chunks, slab_off)
                slab_off += s_chunks * CHUNK

                for cs in range(s_chunks):
                    nod = nod_slab[:, cs * CHUNK : (cs + 1) * CHUNK]
                    mk = mk_slab[
                        :, cs * TILES_PER_CHUNK : (cs + 1) * TILES_PER_CHUNK, :
                    ]

                    if not OPTS["mm1"]:
                        nc.vector.tensor_copy(
                            out=dve_scratch[:1, :1], in_=nod[:1, :1]
                        )
                        nc.vector.tensor_copy(
                            out=dve_scratch[:1, 1:2], in_=mk[:1, 0, :1]
                        )
                        c += 1
                        continue

                    psum_d = ps_pool.tile([P, CHUNK], F32, tag="ps")
                    psum_g = ps_pool.tile([P, CHUNK], F32, tag="ps")
                    for t in range(TILES_PER_CHUNK):
                        sl = bass.ts(t, P)
                        nc.tensor.matmul(
                            psum_d[:, sl], nod[:, sl], wt_sb[:], start=True, stop=True
                        )
                        nc.tensor.matmul(
                            psum_g[:, sl], nod[:, sl], wg_sb[:], start=True, stop=True
                        )

                    if not OPTS["elemwise"]:
                        nc.vector.tensor_copy(
                            out=dve_scratch[:1, :1], in_=psum_d[:1, :1]
                        )
                        nc.vector.tensor_copy(
                            out=dve_scratch[:1, 1:2], in_=psum_g[:1, :1]
                        )
                        nc.vector.tensor_copy(
                            out=dve_scratch[:1, 1:2], in_=mk[:1, 0, :1]
                        )
                        c += 1
                        continue

                    G = OPTS["group"]
                    TPC = TILES_PER_CHUNK
                    if (
                        OPTS["edge_halves"]
                        and G == 1
                        and OPTS["dve_copy_tiles"] == 0
                        and (c == 0 or c == N_CHUNKS - 1)
                    ):
                        # first/last chunk: process in two 4-tile halves so
                        # the serial mm1->TT->sigma->mul->mm2 chain at the
                        # pipeline edges is half as deep (sub-tile deps let
                        # each half start as soon as its mm1s finish)
                        gpre_e = gpre_pool.tile([P, G, CHUNK], F16, tag="gpre")
                        mg_e = d_pool.tile([P, G * TPC, 2 * D], F16, tag="mg")
                        pd3 = psum_d.rearrange("p (t d) -> p t d", d=D)
                        ht = TPC // 2
                        for h in range(2):
                            tlo, thi = h * ht, (h + 1) * ht
                            w0, w1 = tlo * P, thi * P
                            nc.vector.tensor_add(
                                out=gpre_e[:, 0, w0:w1],
                                in0=psum_g[:, w0:w1],
                                in1=bgb_sb[:, w0:w1],
                            )
                            nc.scalar.copy(
                                out=mg_e[:, tlo:thi, :D], in_=pd3[:, tlo:thi, :]
                            )
                            nc.scalar.activation(
                                mg_e[:, tlo:thi, D:],
                                gpre_e[:, 0, w0:w1].rearrange(
                                    "p (t d) -> p t d", d=D
                                ),
                                mybir.ActivationFunctionType.Sigmoid
                                if OPTS["sigmoid"]
                                else mybir.ActivationFunctionType.Copy,
                            )
                            nc.vector.tensor_mul(
                                out=mg_e[:, tlo:thi, :D],
                                in0=mg_e[:, tlo:thi, :D],
                                in1=mg_e[:, tlo:thi, D:],
                            )
                            if OPTS["mm2"]:
                                for t in range(tlo, thi):
                                    first = c == 0 and t == 0
                                    last = c == N_CHUNKS - 1 and t == TPC - 1
                                    nc.tensor.matmul(
                                        pool12[:],
                                        mk[:, t, :],
                                        mg_e[:, t, :],
                                        start=first,
                                        stop=last,
                                        skip_group_check=True,
                                    )
                        c += 1
                        continue
                    gi = c % G
                    if gi == 0:
                        gsize = min(G, N_CHUNKS - c)
                        # group-wide buffers: sigma and the multiply run once
                        # per group to amortize per-op access-latency init
                        gpre_t = gpre_pool.tile([P, G, CHUNK], F16, tag="gpre")
                        mg_t = d_pool.tile([P, G * TPC, 2 * D], F16, tag="mg")
                        pending_mk = []

                    nc.vector.tensor_add(
                        out=gpre_t[:, gi, :], in0=psum_g[:], in1=bgb_sb[:]
                    )
                    psum_d3 = psum_d.rearrange("p (t d) -> p t d", d=D)
                    dct = OPTS["dve_copy_tiles"]
                    split = TPC - dct
                    if split:
                        nc.scalar.copy(
                            out=mg_t[:, gi * TPC : gi * TPC + split, :D],
                            in_=psum_d3[:, :split, :],
                        )
                    if dct:
                        nc.vector.tensor_copy(
                            out=mg_t[:, gi * TPC + split : (gi + 1) * TPC, :D],
                            in_=psum_d3[:, split:, :],
                        )
                    pending_mk.append(mk)

                    if gi == gsize - 1:
                        nt = gsize * TPC
                        nc.scalar.activation(
                            mg_t[:, :nt, D:],
                            gpre_t[:, :gsize, :].rearrange(
                                "p g (t d) -> p (g t) d", d=D
                            ),
                            mybir.ActivationFunctionType.Sigmoid
                            if OPTS["sigmoid"]
                            else mybir.ActivationFunctionType.Copy,
                        )
                        nc.vector.tensor_mul(
                            out=mg_t[:, :nt, :D],
                            in0=mg_t[:, :nt, :D],
                            in1=mg_t[:, :nt, D:],
                        )
                        if OPTS["mm2"]:
                            for pi, pmk in enumerate(pending_mk):
                                cc = c - gsize + 1 + pi
                                for t in range(TPC):
                                    first = cc == 0 and t == 0
                                    last = (
                                        cc == N_CHUNKS - 1 and t == TPC - 1
                                    )
                                    nc.tensor.matmul(
                                        pool12[:],
                                        pmk[:, t, :],
                                        mg_t[:, pi * TPC + t, :],
                                        start=first,
                                        stop=last,
                                        skip_group_check=True,
                                    )
                        else:
                            nc.vector.tensor_copy(
                                out=dve_scratch[:1, :2], in_=mg_t[:1, 0, :2]
                            )
                    c += 1

            res = out_pool.tile([B, 2 * D], F32)
            if OPTS["mm2"]:
                # DVE finishes before ACT at the tail; evicting there starts
                # the output DMA sooner
                nc.vector.tensor_copy(out=res[:], in_=pool12[:])
            else:
                nc.vector.tensor_copy(out=res[:1, :2], in_=dve_scratch[:1, :2])
            nc.sync.dma_start(out_d, res[:])

    return nc


_CACHE: dict = {}


def _get_bass() -> bass.Bass:
    if "nc" not in _CACHE:
        _CACHE["nc"] = build_bass()
    return _CACHE["nc"]


def _prepare_in_maps(nodes, owner_masks, Wt, bt, Wg, bg):
    nodes_h = np.asarray(nodes, dtype=NP_F16)
    masks = np.asarray(owner_masks)
    wt_h = np.ascontiguousarray(np.asarray(Wt, dtype=NP_F16))
    wg_h = np.ascontiguousarray(np.asarray(Wg, dtype=NP_F16))
    bg32 = np.asarray(bg, dtype=np.float32)
    bgb = np.ascontiguousarray(
        np.tile(bg32[None, :], (P, CHUNK // D)).reshape(P, CHUNK)
    )

    in_maps = []
    for core in range(N_CORES):
        off = core * N_PER_CORE
        ncr = np.zeros((P, N_PAD), dtype=NP_F16)
        ncr[:, :N_PER_CORE] = nodes_h[off : off + N_PER_CORE].T
        mp = np.zeros((B, N_PAD), dtype=NP_F16)
        mp[:, :N_PER_CORE] = masks[:, off : off + N_PER_CORE]
        mkt = np.ascontiguousarray(mp.reshape(B, N_TILES, P).transpose(2, 1, 0))
        in_maps.append(
            {
                "nodesT": ncr,
                "masksT": mkt,
                "wt": wt_h,
                "wg": wg_h,
                "bgb": bgb,
            }
        )
    return in_maps


def run(inputs: dict, trace: bool = False):
    """Run the kernel. Returns (pooled [B, D] float32, BassKernelResults)."""
    nc = _get_bass()
    in_maps = _prepare_in_maps(**inputs)
    rb = run_bass_kernel_spmd(
        nc, in_maps, core_ids=list(range(N_CORES)), trace=trace
    )
    parts = np.stack([r["out"].astype(np.float64) for r in rb.results])
    tot = parts.sum(axis=0)
    bt64 = np.asarray(inputs["bt"], dtype=np.float64)
    pooled = tot[:, :D] + tot[:, D:] * bt64[None, :]
    return pooled.astype(np.float32), rb


def kernel(**inputs) -> np.ndarray:
    try:
        out, _ = run(inputs, trace=False)
    except Exception:
        # transient device errors (e.g. residual bad state from a previous
        # crashed NEFF) have been observed once; one retry clears them
        out, _ = run(inputs, trace=False)
    return out


if __name__ == "__main__":
    rng = np.random.default_rng(0)
    demo = {
        "nodes": rng.standard_normal((N_TOTAL, S), dtype=np.float32),
        "owner_masks": rng.integers(0, 2, (B, N_TOTAL)).astype(np.int32),
        "Wt": rng.standard_normal((S, D), dtype=np.float32) * 0.09,
        "bt": rng.standard_normal(D).astype(np.float32) * 0.09,
        "Wg": rng.standard_normal((S, D), dtype=np.float32) * 0.09,
        "bg": rng.standard_normal(D).astype(np.float32) * 0.09,
    }
    out = kernel(**demo)
    print(out.shape, out.dtype, np.abs(out).mean())



# revision 26
# speedup vs baseline: 1.3390x; 1.3390x over previous
"""Trainium2 Bass kernel for nn_Aggregator (gnn_message_passing).

pooled[B,D] = owner_masks.f32 @ ((nodes@Wt + bt) * sigmoid(nodes@Wg + bg))

Sharding: nodes (and owner_masks columns) split along N across 8 cores;
the host sums the 8 partial results and applies the small bt correction
for "W-type" chunks (see below).

Structure (vs the 141.5us v1 kernel):
 - CHUNK = 7 tiles of 128 nodes; psum_d/psum_g are [128, 896] fp32
   (1.75 PSUM banks each) so FOUR rotating psum slots fit alongside the
   [B, 2D] output accumulator -> 2 chunks of pipeline lookahead instead
   of 1.5 (the v2 kernels stalled ~0.5-1.1us per chunk on psum rotation).
 - The gates bias bg enters psum_g via a rank-1 K=1 PE matmul (373ns)
   on most chunks; a fraction uses a DVE tensor_add instead to balance
   PE vs DVE. ACT reads sigmoid straight from PSUM on rank-1 chunks.
 - The data bias bt rides free on the DVE psum_d eviction
   (tensor_tensor add -> fp16). "W-type" chunks instead evict psum_d
   with an ACT copy (no bias) and run a 256-wide mm2 over [msg|G],
   accumulating M@G in pool12[:, D:]; the host applies
   pooled += (M@G)_Wchunks * bt. This trades DVE time for ACT+PE time.
 - msg lives in a combined [128, 7, 256] tile: [:, :, :D] = data/msg,
   [:, :, D:] = gates, so the wide mm2 reads one contiguous 256-col rhs.
 - The fp16 msg*G multiply is split within each chunk: a few tiles on
   DVE (2x mode), the rest on GPSIMD (Pool engine, via engine-field
   retarget of a vector tensor_mul) — parallel halves shorten the
   per-chunk serial chain and offload DVE.
 - mm2 of chunk c is emitted after chunk c+1's mm1 block so the PE
   (in-order) is not stalled by the multiply latency.
"""

import json

import numpy as np

import concourse.bass as bass
import concourse.mybir as mybir
import concourse.tile as tile
from concourse import bass2jax as _b2j
from concourse import bass_utils as _bu
from concourse.bass_utils import run_bass_kernel_spmd


def _split_excess_waits_json(bir_json) -> bytes:
    """Walrus in this container accepts at most 1 embedded sem-wait per
    instruction (2 for EventSemaphore). Tile emits instructions (notably the
    kernel-tail Drain) with more. Move excess waits onto injected
    EventSemaphore instructions placed immediately before the offender in
    the same engine stream — identical blocking semantics."""
    if isinstance(bir_json, str):
        bir_json = bir_json.encode()
    d = json.loads(bir_json)
    counter = [0]

    def fix_block(b):
        new = []
        for inst in b.get("instructions", []):
            si = inst.get("sync_info")
            waits = (si or {}).get("on_wait") or []
            cap = 2 if inst.get("opcode") == "EventSemaphore" else 1
            if len(waits) > cap:
                keep, excess = waits[:cap], waits[cap:]
                for j in range(0, len(excess), 2):
                    counter[0] += 1
                    new.append(
                        {
                            "debug": inst.get("debug"),
                            "engine": inst["engine"],
                            "ins": [],
                            "outs": [],
                            "name": f"antsplit_ev_{counter[0]}",
                            "opcode": "EventSemaphore",
                            "sync_info": {
                                "on_update": [],
                                "on_wait": excess[j : j + 2],
                            },
                        }
                    )
                si["on_wait"] = keep
            new.append(inst)
        b["instructions"] = new
        for sb in b.get("blocks", []):
            fix_block(sb)

    for f in d.get("functions", []):
        for blk in f.get("blocks", []):
            fix_block(blk)
    return json.dumps(d).encode()


if not getattr(_bu, "_ant_split_waits_patched", False):
    _orig_compile_bir_kernel = _bu.compile_bir_kernel

    def _patched_compile_bir_kernel(bir_json, tmpdir, neff_name="file.neff"):
        return _orig_compile_bir_kernel(
            _split_excess_waits_json(bir_json), tmpdir, neff_name
        )

    _bu.compile_bir_kernel = _patched_compile_bir_kernel
    _b2j.compile_bir_kernel = _patched_compile_bir_kernel
    _bu._ant_split_waits_patched = True

N_CORES = 8
N_TOTAL = 500_000
B = 128
S = 128
D = 128
P = 128

N_PER_CORE = N_TOTAL // N_CORES          # 62500
TILES_PER_CHUNK = 8
CHUNK = TILES_PER_CHUNK * P              # 1024
SLAB_CHUNKS = [1, 1, 2] + [4] * 14 + [2]  # 62 chunks
N_CHUNKS = sum(SLAB_CHUNKS)              # 62
N_TILES = N_CHUNKS * TILES_PER_CHUNK     # 496
N_PAD = N_TILES * P                      # 63488

F16 = mybir.dt.float16
F32 = mybir.dt.float32
NP_F16 = np.float16


OPTS = {
    "sigmoid": True,
    "warm_mms": 8,
    "early_nod0": True,
    "touches": True,
    # tiles of each chunk's multiply that run on DVE; the rest go to the
    # Pool (GPSIMD) engine. (dve_tiles, pool_tiles) per chunk parity.
    "mul_dve_tiles": [2, 3],
    "tail_chunks": 3,
    "d_first": True,
    "mm2_delay": 7,
}

# per-chunk mode cycles (index = c % len):
#  d: "tt"  = DVE fused bias-evict (1192ns DVE)
#     "cp"  = W-type: ACT copy + 256-wide mm2, host bt fix (ACT+PE)
#     "cpa" = ACT copy + DVE fp16 bias-add (ACT + 594ns DVE)
#  g: "r1"  = rank-1 PE matmul bias (427ns PE)
#     "tt"  = DVE tensor_add bias (1316ns DVE)
D_CYCLE = ["cp", "tt", "tt", "tt", "tt", "tt", "tt", "tt",
           "tt", "tt", "tt", "tt", "tt", "tt", "tt", "tt"]
G_CYCLE = ["r1", "r1", "r1", "r1", "tt"]


def g_mode(c):
    if c == 0:
        return "r1"  # chunk 0: ones/bgbr arrive before bgb on the DMA ring
    if c >= N_CHUNKS - OPTS["tail_chunks"]:
        return "r1"
    return G_CYCLE[c % len(G_CYCLE)]


def d_mode(c):
    if c == 0:
        return "cp"  # first mm2 (256 wide) initializes the whole pool12
    if c >= N_CHUNKS - OPTS["tail_chunks"]:
        return "tt"
    return D_CYCLE[c % len(D_CYCLE)]


def mul_dve_tiles(c):
    if c >= N_CHUNKS - OPTS["tail_chunks"]:
        return TILES_PER_CHUNK  # tail: all-DVE multiply, short latency
    return OPTS["mul_dve_tiles"][c % len(OPTS["mul_dve_tiles"])]


def build_bass() -> bass.Bass:
    assert sum(SLAB_CHUNKS) == N_CHUNKS
    nc = bass.Bass()

    nodesT = nc.dram_tensor("nodesT", [P, N_PAD], F16, kind="ExternalInput").ap()
    masksT = nc.dram_tensor("masksT", [P, N_TILES, B], F16, kind="ExternalInput").ap()
    wt_d = nc.dram_tensor("wt", [S, D], F16, kind="ExternalInput").ap()
    wg_d = nc.dram_tensor("wg", [S, D], F16, kind="ExternalInput").ap()
    btb_d = nc.dram_tensor("btb", [P, CHUNK], F32, kind="ExternalInput").ap()
    btb16_d = nc.dram_tensor("btb16", [P, CHUNK], F16, kind="ExternalInput").ap()
    bgb_d = nc.dram_tensor("bgb", [P, CHUNK], F32, kind="ExternalInput").ap()
    bgbr_d = nc.dram_tensor("bgbr", [1, CHUNK], F16, kind="ExternalInput").ap()
    ones_d = nc.dram_tensor("ones1", [1, P], F16, kind="ExternalInput").ap()
    out_d = nc.dram_tensor("out", [B, 2 * D], F32, kind="ExternalOutput").ap()

    def pool_mul(out, in0, in1):
        inst = nc.vector.tensor_mul(out=out, in0=in0, in1=in1)
        inst.ins.engine = mybir.EngineType.Pool
        return inst

    with tile.TileContext(nc) as tc:
        with (
            tc.tile_pool(name="consts", bufs=1) as consts,
            tc.tile_pool(name="scratch", bufs=1) as scratch,
            tc.tile_pool(name="nodes", bufs=4) as nodes_pool,
            tc.tile_pool(name="masks", bufs=6) as masks_pool,
            tc.tile_pool(name="gpre", bufs=2) as gpre_pool,
            tc.tile_pool(name="mg", bufs=9) as mg_pool,
            tc.tile_pool(name="outs", bufs=1) as out_pool,
            tc.tile_pool(name="psd", bufs=2, space="PSUM") as psd_pool,
            tc.tile_pool(name="psg", bufs=3, space="PSUM") as psg_pool,
            tc.tile_pool(name="acc", bufs=1, space="PSUM") as acc_pool,
        ):
            def emit_slab(s_chunks, slab_off):
                slab_n = s_chunks * CHUNK
                nod_slab = nodes_pool.tile([P, 4 * CHUNK], F16, tag="nod_slab")
                nc.sync.dma_start(
                    nod_slab[:, :slab_n],
                    nodesT[:, slab_off : slab_off + slab_n],
                )
                mk_slab = masks_pool.tile(
                    [P, 4 * TILES_PER_CHUNK, B], F16, tag="mk_slab"
                )
                to = slab_off // P
                nc.sync.dma_start(
                    mk_slab[:, : s_chunks * TILES_PER_CHUNK, :],
                    masksT[:, to : to + s_chunks * TILES_PER_CHUNK, :],
                )
                return nod_slab, mk_slab

            wt_sb = consts.tile([S, D], F16)
            nc.sync.dma_start(wt_sb[:], wt_d)
            if OPTS["early_nod0"]:
                # put the first node slab on the DMA ring right after wt so
                # its (large) transfer overlaps the remaining const loads
                wg_sb = consts.tile([S, D], F16)
                nc.sync.dma_start(wg_sb[:], wg_d)
                slab_n0 = SLAB_CHUNKS[0] * CHUNK
                nod_slab0 = nodes_pool.tile([P, 4 * CHUNK], F16, tag="nod_slab")
                nc.sync.dma_start(nod_slab0[:, :slab_n0], nodesT[:, :slab_n0])
            if not OPTS["early_nod0"]:
                wg_sb = consts.tile([S, D], F16)
                nc.sync.dma_start(wg_sb[:], wg_d)
            btb_sb = consts.tile([P, CHUNK], F32)
            nc.sync.dma_start(btb_sb[:], btb_d)
            btb16_sb = consts.tile([P, CHUNK], F16)
            nc.sync.dma_start(btb16_sb[:], btb16_d)
            bgb_sb = consts.tile([P, CHUNK], F32)
            nc.sync.dma_start(bgb_sb[:], bgb_d)
            bgbr_sb = consts.tile([1, CHUNK], F16)
            nc.sync.dma_start(bgbr_sb[:], bgbr_d)
            ones_sb = consts.tile([1, P], F16)
            nc.sync.dma_start(ones_sb[:], ones_d)

            # One-time const touches: absorb the const-DMA semaphores into
            # each engine's observed clock so hot-loop instructions never
            # need a second (DMA) wait slot.
            if OPTS["touches"]:
                dve_scratch = scratch.tile([1, 4], F32)
                nc.vector.tensor_copy(out=dve_scratch[:1, :1], in_=btb_sb[:1, :1])
                nc.vector.tensor_copy(out=dve_scratch[:1, 1:2], in_=bgb_sb[:1, :1])
                pool_scr = scratch.tile([1, 4], F16, tag="pscr")
                pool_mul(pool_scr[:1, :1], bgbr_sb[:1, :1], bgbr_sb[:1, :1])
                nc.tensor.ldweights(wt_sb[:, :1])
                nc.tensor.ldweights(wg_sb[:, :1])
                nc.tensor.ldweights(ones_sb[:, :1])
            if OPTS["warm_mms"]:
                # burn the PE HAM clock ramp during the initial DMA wait;
                # scratch lives in the rotating psum pool (one-time slot use)
                warm_ps = psg_pool.tile([P, CHUNK // 2], F32, tag="psg")
                for _ in range(OPTS["warm_mms"]):
                    nc.tensor.matmul(
                        warm_ps[:, :D], wt_sb[:], wg_sb[:], start=True, stop=True
                    )
                nc.vector.tensor_copy(
                    out=dve_scratch[:1, 2:3], in_=warm_ps[:1, :1]
                )
            if OPTS["early_nod0"]:
                mk_slab0 = masks_pool.tile(
                    [P, 4 * TILES_PER_CHUNK, B], F16, tag="mk_slab"
                )
                nc.sync.dma_start(
                    mk_slab0[:, : SLAB_CHUNKS[0] * TILES_PER_CHUNK, :],
                    masksT[:, : SLAB_CHUNKS[0] * TILES_PER_CHUNK, :],
                )
                slab0 = (nod_slab0, mk_slab0)
            else:
                slab0 = emit_slab(SLAB_CHUNKS[0], 0)
            # chunk 0 is (r1, cp): needs ones/bgbr early; they are tiny and
            # follow the first slabs on the ring

            # pool12[:, :D] accumulates M@msg; [:, D:] accumulates M@G for
            # W-type chunks only (host multiplies by bt). Chunk 0 is W-type
            # and its first 256-wide mm2 carries start=True for the whole
            # [B, 2D] region.
            pool12 = acc_pool.tile([B, 2 * D], F32)

            def emit_mm2(mk, mg_t, cc):
                wide = d_mode(cc) == "cp"
                for t in range(TILES_PER_CHUNK):
                    first = cc == 0 and t == 0
                    last = cc == N_CHUNKS - 1 and t == TILES_PER_CHUNK - 1
                    nc.tensor.matmul(
                        pool12[:] if wide else pool12[:, :D],
                        mk[:, t, :],
                        mg_t[:, t, :] if wide else mg_t[:, t, :D],
                        start=first,
                        stop=last,
                        skip_group_check=True,
                    )

            pending_mm2 = []  # [(mk, mg_t, c), ...] delayed by two chunks
            c = 0
            slab_off = 0
            for s, s_chunks in enumerate(SLAB_CHUNKS):
                if s == 0:
                    nod_slab, mk_slab = slab0
                else:
                    nod_slab, mk_slab = emit_slab(s_chunks, slab_off)
                slab_off += s_chunks * CHUNK

                for cs in range(s_chunks):
                    nod = nod_slab[:, cs * CHUNK : (cs + 1) * CHUNK]
                    mk = mk_slab[
                        :, cs * TILES_PER_CHUNK : (cs + 1) * TILES_PER_CHUNK, :
                    ]
                    gm = g_mode(c)
                    dm = d_mode(c)

                    H = CHUNK // 2
                    HT = TILES_PER_CHUNK // 2
                    psum_d = psd_pool.tile([P, CHUNK], F32, tag="psd")
                    pg0 = psg_pool.tile([P, H], F32, tag="psg")
                    pg1 = psg_pool.tile([P, H], F32, tag="psg")
                    psum_g = [pg0, pg1]
                    if gm == "r1":
                        for pg in psum_g:
                            nc.tensor.matmul(
                                pg[:],
                                ones_sb[:],
                                bgbr_sb[:, :H],
                                start=True,
                                stop=False,
                                skip_group_check=True,
                            )
                    for t in range(TILES_PER_CHUNK):
                        sl = bass.ts(t, P)
                        mmd = lambda: nc.tensor.matmul(
                            psum_d[:, sl], nod[:, sl], wt_sb[:], start=True, stop=True
                        )
                        mmg = lambda: nc.tensor.matmul(
                            psum_g[t // HT][:, bass.ts(t % HT, P)],
                            nod[:, sl],
                            wg_sb[:],
                            start=gm != "r1",
                            stop=True,
                            skip_group_check=gm == "r1",
                        )
                        if OPTS["d_first"]:
                            mmd(); mmg()
                        else:
                            mmg(); mmd()

                    # an earlier chunk's pooling matmuls go here in the PE
                    # stream: its multiply gets two chunks of mm1 as slack
                    if len(pending_mm2) >= OPTS["mm2_delay"]:
                        emit_mm2(*pending_mm2.pop(0))

                    mg_t = mg_pool.tile([P, TILES_PER_CHUNK, 2 * D], F16, tag="mg")
                    sig = (
                        mybir.ActivationFunctionType.Sigmoid
                        if OPTS["sigmoid"]
                        else mybir.ActivationFunctionType.Copy
                    )
                    psum_d3 = psum_d.rearrange("p (t d) -> p t d", d=D)
                    # data-path eviction first when it runs on ACT (in-order
                    # engine; psum_d is ready before the gates matmuls)
                    if dm in ("cp", "cpa"):
                        nc.scalar.copy(out=mg_t[:, :, :D], in_=psum_d3[:])
                    # gates into mg[:, :, D:]
                    if gm == "r1":
                        for h, pg in enumerate(psum_g):
                            nc.scalar.activation(
                                mg_t[:, h * HT : (h + 1) * HT, D:],
                                pg.rearrange("p (t d) -> p t d", d=D),
                                sig,
                            )
                    else:
                        gpre_t = gpre_pool.tile([P, CHUNK], F16, tag="gpre")
                        for h, pg in enumerate(psum_g):
                            nc.vector.tensor_add(
                                out=gpre_t[:, h * H : (h + 1) * H],
                                in0=pg[:],
                                in1=bgb_sb[:, :H],
                            )
                        nc.scalar.activation(
                            mg_t[:, :, D:],
                            gpre_t.rearrange("p (t d) -> p t d", d=D),
                            sig,
                        )
                    # data into mg[:, :, :D] (cp: already ACT-copied above,
                    # bias applied via the host fix; cpa: fp16 bias add)
                    if dm == "tt":
                        nc.vector.tensor_add(
                            out=mg_t[:, :, :D],
                            in0=psum_d3[:],
                            in1=btb_sb.rearrange("p (t d) -> p t d", d=D),
                        )
                    elif dm == "cpa":
                        nc.vector.tensor_add(
                            out=mg_t[:, :, :D],
                            in0=mg_t[:, :, :D],
                            in1=btb16_sb.rearrange("p (t d) -> p t d", d=D),
                        )
                    # multiply msg = a * g, split DVE/Pool within the chunk
                    ndve = mul_dve_tiles(c)
                    if ndve > 0:
                        nc.vector.tensor_mul(
                            out=mg_t[:, :ndve, :D],
                            in0=mg_t[:, :ndve, :D],
                            in1=mg_t[:, :ndve, D:],
                        )
                    if ndve < HT:
                        # two Pool ops aligned with the sigmoid halves so
                        # each starts as soon as its gates half is ready
                        pool_mul(
                            mg_t[:, ndve:HT, :D],
                            mg_t[:, ndve:HT, :D],
                            mg_t[:, ndve:HT, D:],
                        )
                    if ndve < TILES_PER_CHUNK:
                        pool_mul(
                            mg_t[:, max(ndve, HT):, :D],
                            mg_t[:, max(ndve, HT):, :D],
                            mg_t[:, max(ndve, HT):, D:],
                        )

                    pending_mm2.append((mk, mg_t, c))
                    c += 1

            for pm in pending_mm2:
                emit_mm2(*pm)
            pending_mm2 = []

            res = out_pool.tile([B, 2 * D], F32)
            nc.vector.tensor_copy(out=res[:], in_=pool12[:])
            nc.sync.dma_start(out_d, res[:])

    return nc


_CACHE: dict = {}


def _get_bass() -> bass.Bass:
    if "nc" not in _CACHE:
        _CACHE["nc"] = build_bass()
    return _CACHE["nc"]


def _prepare_in_maps(nodes, owner_masks, Wt, bt, Wg, bg):
    nodes_h = np.asarray(nodes, dtype=NP_F16)
    masks = np.asarray(owner_masks)
    wt_h = np.ascontiguousarray(np.asarray(Wt, dtype=NP_F16))
    wg_h = np.ascontiguousarray(np.asarray(Wg, dtype=NP_F16))
    bt32 = np.asarray(bt, dtype=np.float32)
    bg32 = np.asarray(bg, dtype=np.float32)

    btb = np.ascontiguousarray(
        np.tile(bt32[None, :], (P, CHUNK // D)).reshape(P, CHUNK)
    )
    btb16 = np.ascontiguousarray(
        np.tile(bt32.astype(NP_F16)[None, :], (P, CHUNK // D)).reshape(P, CHUNK)
    )
    bgb = np.ascontiguousarray(
        np.tile(bg32[None, :], (P, CHUNK // D)).reshape(P, CHUNK)
    )
    bgbr = np.ascontiguousarray(
        np.tile(bg32.astype(NP_F16)[None, :], (1, CHUNK // D)).reshape(1, CHUNK)
    )
    ones1 = np.ones((1, P), dtype=NP_F16)

    in_maps = []
    for core in range(N_CORES):
        off = core * N_PER_CORE
        ncr = np.zeros((P, N_PAD), dtype=NP_F16)
        ncr[:, :N_PER_CORE] = nodes_h[off : off + N_PER_CORE].T
        mp = np.zeros((B, N_PAD), dtype=NP_F16)
        mp[:, :N_PER_CORE] = masks[:, off : off + N_PER_CORE]
        mkt = np.ascontiguousarray(mp.reshape(B, N_TILES, P).transpose(2, 1, 0))
        in_maps.append(
            {
                "nodesT": ncr,
                "masksT": mkt,
                "wt": wt_h,
                "wg": wg_h,
                "btb": btb,
                "btb16": btb16,
                "bgb": bgb,
                "bgbr": bgbr,
                "ones1": ones1,
            }
        )
    return in_maps


def run(inputs: dict, trace: bool = False):
    """Run the kernel. Returns (pooled [B, D] float32, BassKernelResults)."""
    nc = _get_bass()
    in_maps = _prepare_in_maps(**inputs)
    rb = run_bass_kernel_spmd(
        nc, in_maps, core_ids=list(range(N_CORES)), trace=trace
    )
    parts = np.stack([r["out"].astype(np.float64) for r in rb.results])
    tot = parts.sum(axis=0)
    bt64 = np.asarray(inputs["bt"], dtype=np.float64)
    pooled = tot[:, :D] + tot[:, D:] * bt64[None, :]
    return pooled.astype(np.float32), rb


def kernel(**inputs) -> np.ndarray:
    try:
        out, _ = run(inputs, trace=False)
    except Exception:
        # transient device errors (e.g. residual bad state from a previous
        # crashed NEFF) have been observed once; one retry clears them
        out, _ = run(inputs, trace=False)
    return out


if __name__ == "__main__":
    rng = np.random.default_rng(0)
    demo = {
        "nodes": rng.standard_normal((N_TOTAL, S), dtype=np.float32),
        "owner_masks": rng.integers(0, 2, (B, N_TOTAL)).astype(np.int32),
        "Wt": rng.standard_normal((S, D), dtype=np.float32) * 0.09,
        "bt": rng.standard_normal(D).astype(np.float32) * 0.09,
        "Wg": rng.standard_normal((S, D), dtype=np.float32) * 0.09,
        "bg": rng.standard_normal(D).astype(np.float32) * 0.09,
    }
    out = kernel(**demo)
    print(out.shape, out.dtype, np.abs(out).mean())


# revision 38
# speedup vs baseline: 2.5110x; 1.8753x over previous
"""Trainium2 Bass kernel for nn_Aggregator (gnn_message_passing).

pooled[B,D] = owner_masks.f32 @ ((nodes@Wt + bt) * sigmoid(nodes@Wg + bg))

Sharding: nodes (and owner_masks columns) split along N across 8 cores;
the host sums the 8 partial results and applies the small bt correction
for "W-type" chunks (see below).

Structure (vs the 141.5us v1 kernel):
 - Per chunk (8 tiles of 128 nodes): 16 mm1 matmuls -> psum_d [128,1024]
   and psum_g split into two [128,512] half-tiles. Asymmetric PSUM pools
   (psum_d 2 bufs x 2 banks, psum_g halves 3 bufs x 1 bank, [B,2D]
   accumulator 1 bank = 8 banks) give 2 chunks of rotation lookahead.
 - The gates bias bg enters psum_g via rank-1 K=1 PE matmuls (427ns)
   on ~80% of chunks; the rest use a DVE tensor_add into a gpre tile to
   balance PE vs DVE. ACT reads sigmoid straight from PSUM on rank-1
   chunks (one op per half, freeing each psum_g half early).
 - The data bias bt rides free on the DVE psum_d eviction
   (tensor_tensor add psum fp32 + fp16 btb -> fp16). One "W-type" chunk
   (c=0) instead evicts with an ACT copy and runs a 256-wide mm2 over
   [msg|G], accumulating M@G in pool12[:, D:]; the host applies
   pooled += (M@G)_W * bt. Its first mm2 initializes pool12.
 - msg lives in a combined [128, 8, 256] tile ([:, :, :D] msg,
   [:, :, D:] gates); mm2 reads [B,128] rhs slices ([B,256] for W).
 - The fp16 msg*G multiply is split within each chunk: 2-3 tiles on DVE
   (2x mode) and the rest on GPSIMD (Pool engine, via engine-field
   retarget of a vector tensor_mul) in two ops aligned with the sigmoid
   halves. This offloads DVE and shortens the serial chain.
 - mm2 of chunk c is emitted ~7 chunks later in the PE stream so the
   (in-order) PE is never stalled by the multiply latency.
 - All fp16 constants ship in two packed DMAs (each dma_start costs
   650ns of serial dispatch on the SP queue); mask slabs are enqueued
   one slab behind node slabs since mm2 trails by 7 chunks. This gets
   first compute ~3us earlier.
"""

import json

import numpy as np

import concourse.bass as bass
import concourse.mybir as mybir
import concourse.tile as tile
from concourse import bass2jax as _b2j
from concourse import bass_utils as _bu
from concourse.bass_utils import run_bass_kernel_spmd


def _split_excess_waits_json(bir_json) -> bytes:
    """Walrus in this container accepts at most 1 embedded sem-wait per
    instruction (2 for EventSemaphore). Tile emits instructions (notably the
    kernel-tail Drain) with more. Move excess waits onto injected
    EventSemaphore instructions placed immediately before the offender in
    the same engine stream — identical blocking semantics."""
    if isinstance(bir_json, str):
        bir_json = bir_json.encode()
    d = json.loads(bir_json)
    counter = [0]

    def fix_block(b):
        new = []
        for inst in b.get("instructions", []):
            si = inst.get("sync_info")
            waits = (si or {}).get("on_wait") or []
            cap = 2 if inst.get("opcode") == "EventSemaphore" else 1
            if len(waits) > cap:
                keep, excess = waits[:cap], waits[cap:]
                for j in range(0, len(excess), 2):
                    counter[0] += 1
                    new.append(
                        {
                            "debug": inst.get("debug"),
                            "engine": inst["engine"],
                            "ins": [],
                            "outs": [],
                            "name": f"antsplit_ev_{counter[0]}",
                            "opcode": "EventSemaphore",
                            "sync_info": {
                                "on_update": [],
                                "on_wait": excess[j : j + 2],
                            },
                        }
                    )
                si["on_wait"] = keep
            new.append(inst)
        b["instructions"] = new
        for sb in b.get("blocks", []):
            fix_block(sb)

    for f in d.get("functions", []):
        for blk in f.get("blocks", []):
            fix_block(blk)
    return json.dumps(d).encode()


if not getattr(_bu, "_ant_split_waits_patched", False):
    _orig_compile_bir_kernel = _bu.compile_bir_kernel

    def _patched_compile_bir_kernel(bir_json, tmpdir, neff_name="file.neff"):
        return _orig_compile_bir_kernel(
            _split_excess_waits_json(bir_json), tmpdir, neff_name
        )

    _bu.compile_bir_kernel = _patched_compile_bir_kernel
    _b2j.compile_bir_kernel = _patched_compile_bir_kernel
    _bu._ant_split_waits_patched = True

N_CORES = 8
N_TOTAL = 500_000
B = 128
S = 128
D = 128
P = 128

N_PER_CORE = N_TOTAL // N_CORES          # 62500
TILES_PER_CHUNK = 8
CHUNK = TILES_PER_CHUNK * P              # 1024
SLAB_CHUNKS = [1, 1, 2] + [4] * 14 + [2]  # 62 chunks
N_CHUNKS = sum(SLAB_CHUNKS)              # 62
N_TILES = N_CHUNKS * TILES_PER_CHUNK     # 496
N_PAD = N_TILES * P                      # 63488

F16 = mybir.dt.float16
F32 = mybir.dt.float32
NP_F16 = np.float16

# packed fp16 const layout (one [P, CW16] dram tensor / SBUF tile):
#   cols 0:1024       btb16 (bt tiled, all partitions)
#   cols 1024:1152    wt
#   cols 1152:1280    wg
#   cols 1280:2304    row p=0: bgbr (bg tiled)
#   cols 2304:3328    bgb16 (bg tiled, all partitions)
#   cols 3328:3456    row p=0: ones
CW16 = 3456

OPTS = {
    "sigmoid": True,
    "warm_mms": 8,
    "touches": True,
    # tiles of each chunk's multiply that run on DVE; the rest go to the
    # Pool (GPSIMD) engine
    "mul_dve_tiles": [2, 3],
    "tail_chunks": 3,
    "d_first": True,
    "mm2_delay": 7,
}

# per-chunk mode cycles (index = c % len):
#  d: "tt" = DVE fused bias-evict; "cp" = W-type (ACT copy evict,
#     256-wide mm2, host bt fix)
#  g: "r1" = rank-1 PE matmul bias; "tt" = DVE tensor_add bias
D_CYCLE = ["cp", "tt", "tt", "tt", "tt", "tt", "tt", "tt",
           "tt", "tt", "tt", "tt", "tt", "tt", "tt", "tt"]
G_CYCLE = ["r1", "r1", "r1", "r1", "tt"]


def g_mode(c):
    if c == 0:
        return "r1"
    if c >= N_CHUNKS - OPTS["tail_chunks"]:
        return "r1"
    return G_CYCLE[c % len(G_CYCLE)]


def d_mode(c):
    if c == 0:
        return "cp"  # first mm2 (256 wide) initializes the whole pool12
    if c >= N_CHUNKS - OPTS["tail_chunks"]:
        return "tt"
    return D_CYCLE[c % len(D_CYCLE)]


def mul_dve_tiles(c):
    if c >= N_CHUNKS - OPTS["tail_chunks"]:
        return TILES_PER_CHUNK  # tail: all-DVE multiply, short latency
    return OPTS["mul_dve_tiles"][c % len(OPTS["mul_dve_tiles"])]


def build_bass() -> bass.Bass:
    assert sum(SLAB_CHUNKS) == N_CHUNKS
    nc = bass.Bass()

    nodesT = nc.dram_tensor("nodesT", [P, N_PAD], F16, kind="ExternalInput").ap()
    masksT = nc.dram_tensor("masksT", [P, N_TILES, B], F16, kind="ExternalInput").ap()
    c16_d = nc.dram_tensor("c16", [P, CW16], F16, kind="ExternalInput").ap()
    out_d = nc.dram_tensor("out", [B, 2 * D], F32, kind="ExternalOutput").ap()

    def pool_mul(out, in0, in1):
        inst = nc.vector.tensor_mul(out=out, in0=in0, in1=in1)
        inst.ins.engine = mybir.EngineType.Pool
        return inst

    nslabs = len(SLAB_CHUNKS)
    slab_off = [0] * nslabs
    off = 0
    for s, sc in enumerate(SLAB_CHUNKS):
        slab_off[s] = off
        off += sc * CHUNK

    with tile.TileContext(nc) as tc:
        with (
            tc.tile_pool(name="consts", bufs=1) as consts,
            tc.tile_pool(name="scratch", bufs=1) as scratch,
            tc.tile_pool(name="nodes", bufs=4) as nodes_pool,
            tc.tile_pool(name="masks", bufs=6) as masks_pool,
            tc.tile_pool(name="gpre", bufs=2) as gpre_pool,
            tc.tile_pool(name="mg", bufs=9) as mg_pool,
            tc.tile_pool(name="outs", bufs=1) as out_pool,
            tc.tile_pool(name="psd", bufs=2, space="PSUM") as psd_pool,
            tc.tile_pool(name="psg", bufs=3, space="PSUM") as psg_pool,
            tc.tile_pool(name="acc", bufs=1, space="PSUM") as acc_pool,
        ):
            def nodes_tile():
                return nodes_pool.tile(
                    [P, 4 * CHUNK], F16, tag="nod_slab", name="nod_slab"
                )

            def masks_tile():
                return masks_pool.tile(
                    [P, 4 * TILES_PER_CHUNK, B], F16,
                    tag="mk_slab", name="mk_slab",
                )

            def emit_nodes_dma(tile_, s):
                nc.sync.dma_start(
                    tile_[:, : SLAB_CHUNKS[s] * CHUNK],
                    nodesT[:, slab_off[s] : slab_off[s] + SLAB_CHUNKS[s] * CHUNK],
                )

            def emit_masks_dma(tile_, s):
                to = slab_off[s] // P
                nt = SLAB_CHUNKS[s] * TILES_PER_CHUNK
                nc.sync.dma_start(tile_[:, :nt, :], masksT[:, to : to + nt, :])

            # ---- startup DMA queue: wt/wg/bgbr/ones block, first nodes,
            # remaining consts; masks trail nodes by one slab ----
            c16_sb = consts.tile([P, CW16], F16)
            nc.sync.dma_start(c16_sb[:, 1024:2304], c16_d[:, 1024:2304])  # wt|wg|bgbr
            nod_slabs = [nodes_tile()]
            emit_nodes_dma(nod_slabs[0], 0)
            nc.sync.dma_start(c16_sb[:, :1024], c16_d[:, :1024])
            nc.sync.dma_start(c16_sb[:, 2304:], c16_d[:, 2304:])  # bgb16+ones

            btb16_sb = c16_sb[:, 0:1024]
            wt_sb = c16_sb[:, 1024:1152]
            wg_sb = c16_sb[:, 1152:1280]
            bgbr_sb = c16_sb[0:1, 1280:2304]
            ones_sb = c16_sb[0:1, 3328:3456]
            bgb16_sb = c16_sb[:, 2304:3328]

            # One-time const touches: absorb the const-DMA semaphores into
            # each engine's observed clock so hot-loop instructions never
            # need a second (DMA) wait slot.
            if OPTS["touches"]:
                dve_scratch = scratch.tile([1, 4], F32)
                nc.vector.tensor_copy(
                    out=dve_scratch[:1, :1], in_=btb16_sb[:1, :1]
                )
                nc.vector.tensor_copy(
                    out=dve_scratch[:1, 1:2], in_=bgb16_sb[:1, :1]
                )
                pool_scr = scratch.tile([1, 4], F16, tag="pscr")
                pool_mul(pool_scr[:1, :1], bgbr_sb[:1, :1], bgbr_sb[:1, :1])
                nc.tensor.ldweights(wt_sb[:, :1])
                nc.tensor.ldweights(wg_sb[:, :1])
                nc.tensor.ldweights(ones_sb[:, :1])
            if OPTS["warm_mms"]:
                # burn the PE clock ramp during the initial DMA wait
                warm_ps = psg_pool.tile([P, CHUNK // 2], F32, tag="psg")
                for _ in range(OPTS["warm_mms"]):
                    nc.tensor.matmul(
                        warm_ps[:, :D], wt_sb[:], wg_sb[:], start=True, stop=True
                    )
                nc.vector.tensor_copy(
                    out=dve_scratch[:1, 2:3], in_=warm_ps[:1, :1]
                )

            # pool12[:, :D] accumulates M@msg; [:, D:] accumulates M@G for
            # W-type chunks only (host multiplies by bt). Chunk 0 is W-type
            # and its first 256-wide mm2 carries start=True for the whole
            # [B, 2D] region.
            pool12 = acc_pool.tile([B, 2 * D], F32)

            def emit_mm2(mk, mg_t, cc):
                wide = d_mode(cc) == "cp"
                for t in range(TILES_PER_CHUNK):
                    first = cc == 0 and t == 0
                    last = cc == N_CHUNKS - 1 and t == TILES_PER_CHUNK - 1
                    nc.tensor.matmul(
                        pool12[:] if wide else pool12[:, :D],
                        mk[:, t, :],
                        mg_t[:, t, :] if wide else mg_t[:, t, :D],
                        start=first,
                        stop=last,
                        skip_group_check=True,
                    )

            pending_mm2 = []  # [(mk, mg_t, c), ...] delayed by mm2_delay
            c = 0
            mk_slabs = [masks_tile()]
            for s in range(nslabs):
                # prefetch next slab's nodes; this slab's masks DMA
                # (mask tiles pre-allocated so chunk code can reference them)
                if s + 1 < nslabs:
                    nt = nodes_tile()
                    emit_nodes_dma(nt, s + 1)
                    nod_slabs.append(nt)
                emit_masks_dma(mk_slabs[s], s)
                if s + 1 < nslabs:
                    mk_slabs.append(masks_tile())
                nod_slab = nod_slabs[s]
                mk_slab = mk_slabs[s]

                for cs in range(SLAB_CHUNKS[s]):
                    nod = nod_slab[:, cs * CHUNK : (cs + 1) * CHUNK]
                    mk = mk_slab[
                        :, cs * TILES_PER_CHUNK : (cs + 1) * TILES_PER_CHUNK, :
                    ]
                    gm = g_mode(c)
                    dm = d_mode(c)

                    H = CHUNK // 2
                    HT = TILES_PER_CHUNK // 2
                    psum_d = psd_pool.tile([P, CHUNK], F32, tag="psd")
                    pg0 = psg_pool.tile([P, H], F32, tag="psg")
                    pg1 = psg_pool.tile([P, H], F32, tag="psg")
                    psum_g = [pg0, pg1]
                    if gm == "r1":
                        for pg in psum_g:
                            nc.tensor.matmul(
                                pg[:],
                                ones_sb[:],
                                bgbr_sb[:, :H],
                                start=True,
                                stop=False,
                                skip_group_check=True,
                            )
                    for t in range(TILES_PER_CHUNK):
                        sl = bass.ts(t, P)

                        def mmd():
                            nc.tensor.matmul(
                                psum_d[:, sl],
                                nod[:, sl],
                                wt_sb[:],
                                start=True,
                                stop=True,
                            )

                        def mmg():
                            nc.tensor.matmul(
                                psum_g[t // HT][:, bass.ts(t % HT, P)],
                                nod[:, sl],
                                wg_sb[:],
                                start=gm != "r1",
                                stop=True,
                                skip_group_check=gm == "r1",
                            )

                        if OPTS["d_first"]:
                            mmd()
                            mmg()
                        else:
                            mmg()
                            mmd()

                    # an earlier chunk's pooling matmuls go here in the PE
                    # stream: its multiply gets several chunks of mm1 slack
                    if len(pending_mm2) >= OPTS["mm2_delay"]:
                        emit_mm2(*pending_mm2.pop(0))

                    mg_t = mg_pool.tile(
                        [P, TILES_PER_CHUNK, 2 * D], F16, tag="mg"
                    )
                    sig = (
                        mybir.ActivationFunctionType.Sigmoid
                        if OPTS["sigmoid"]
                        else mybir.ActivationFunctionType.Copy
                    )
                    psum_d3 = psum_d.rearrange("p (t d) -> p t d", d=D)
                    # data-path eviction first when it runs on ACT (in-order
                    # engine; psum_d is ready before the gates matmuls)
                    if dm == "cp":
                        nc.scalar.copy(out=mg_t[:, :, :D], in_=psum_d3[:])
                    # gates into mg[:, :, D:]
                    if gm == "r1":
                        for h, pg in enumerate(psum_g):
                            nc.scalar.activation(
                                mg_t[:, h * HT : (h + 1) * HT, D:],
                                pg.rearrange("p (t d) -> p t d", d=D),
                                sig,
                            )
                    else:
                        gpre_t = gpre_pool.tile([P, CHUNK], F16, tag="gpre")
                        for h, pg in enumerate(psum_g):
                            nc.vector.tensor_add(
                                out=gpre_t[:, h * H : (h + 1) * H],
                                in0=pg[:],
                                in1=bgb16_sb[:, :H],
                            )
                        nc.scalar.activation(
                            mg_t[:, :, D:],
                            gpre_t.rearrange("p (t d) -> p t d", d=D),
                            sig,
                        )
                    # data into mg[:, :, :D] (cp: copied above, host bt fix)
                    if dm == "tt":
                        nc.vector.tensor_add(
                            out=mg_t[:, :, :D],
                            in0=psum_d3[:],
                            in1=btb16_sb.rearrange("p (t d) -> p t d", d=D),
                        )
                    # multiply msg = a * g, split DVE/Pool within the chunk
                    ndve = mul_dve_tiles(c)
                    if ndve > 0:
                        nc.vector.tensor_mul(
                            out=mg_t[:, :ndve, :D],
                            in0=mg_t[:, :ndve, :D],
                            in1=mg_t[:, :ndve, D:],
                        )
                    if ndve < HT:
                        # two Pool ops aligned with the sigmoid halves so
                        # each starts as soon as its gates half is ready
                        pool_mul(
                            mg_t[:, ndve:HT, :D],
                            mg_t[:, ndve:HT, :D],
                            mg_t[:, ndve:HT, D:],
                        )
                    if ndve < TILES_PER_CHUNK:
                        pool_mul(
                            mg_t[:, max(ndve, HT) :, :D],
                            mg_t[:, max(ndve, HT) :, :D],
                            mg_t[:, max(ndve, HT) :, D:],
                        )

                    pending_mm2.append((mk, mg_t, c))
                    c += 1

            for pm in pending_mm2:
                emit_mm2(*pm)
            pending_mm2 = []

            res = out_pool.tile([B, 2 * D], F32)
            nc.vector.tensor_copy(out=res[:], in_=pool12[:])
            nc.sync.dma_start(out_d, res[:])

    return nc


_CACHE: dict = {}


def _get_bass() -> bass.Bass:
    if "nc" not in _CACHE:
        _CACHE["nc"] = build_bass()
    return _CACHE["nc"]


def _prepare_in_maps(nodes, owner_masks, Wt, bt, Wg, bg):
    nodes_h = np.asarray(nodes, dtype=NP_F16)
    masks = np.asarray(owner_masks)
    wt_h = np.asarray(Wt, dtype=NP_F16)
    wg_h = np.asarray(Wg, dtype=NP_F16)
    bt16 = np.asarray(bt, dtype=NP_F16)
    bg16 = np.asarray(bg, dtype=NP_F16)

    c16 = np.zeros((P, CW16), dtype=NP_F16)
    c16[:, 0:1024] = np.tile(bt16[None, :], (P, CHUNK // D))
    c16[:, 1024:1152] = wt_h
    c16[:, 1152:1280] = wg_h
    c16[0, 1280:2304] = np.tile(bg16, CHUNK // D)
    c16[0, 3328:3456] = 1.0
    c16[:, 2304:3328] = np.tile(bg16[None, :], (P, CHUNK // D))

    in_maps = []
    for core in range(N_CORES):
        off = core * N_PER_CORE
        ncr = np.zeros((P, N_PAD), dtype=NP_F16)
        ncr[:, :N_PER_CORE] = nodes_h[off : off + N_PER_CORE].T
        mp = np.zeros((B, N_PAD), dtype=NP_F16)
        mp[:, :N_PER_CORE] = masks[:, off : off + N_PER_CORE]
        mkt = np.ascontiguousarray(mp.reshape(B, N_TILES, P).transpose(2, 1, 0))
        in_maps.append(
            {
                "nodesT": ncr,
                "masksT": mkt,
                "c16": c16,
            }
        )
    return in_maps


def run(inputs: dict, trace: bool = False):
    """Run the kernel. Returns (pooled [B, D] float32, BassKernelResults)."""
    nc = _get_bass()
    in_maps = _prepare_in_maps(**inputs)
    rb = run_bass_kernel_spmd(
        nc, in_maps, core_ids=list(range(N_CORES)), trace=trace
    )
    parts = np.stack([r["out"].astype(np.float64) for r in rb.results])
    tot = parts.sum(axis=0)
    bt64 = np.asarray(inputs["bt"], dtype=np.float64)
    pooled = tot[:, :D] + tot[:, D:] * bt64[None, :]
    return pooled.astype(np.float32), rb


def kernel(**inputs) -> np.ndarray:
    try:
        out, _ = run(inputs, trace=False)
    except Exception:
        # transient device errors (e.g. residual bad state from a previous
        # crashed NEFF) have been observed once; one retry clears them
        out, _ = run(inputs, trace=False)
    return out


if __name__ == "__main__":
    rng = np.random.default_rng(0)
    demo = {
        "nodes": rng.standard_normal((N_TOTAL, S), dtype=np.float32),
        "owner_masks": rng.integers(0, 2, (B, N_TOTAL)).astype(np.int32),
        "Wt": rng.standard_normal((S, D), dtype=np.float32) * 0.09,
        "bt": rng.standard_normal(D).astype(np.float32) * 0.09,
        "Wg": rng.standard_normal((S, D), dtype=np.float32) * 0.09,
        "bg": rng.standard_normal(D).astype(np.float32) * 0.09,
    }
    out = kernel(**demo)
    print(out.shape, out.dtype, np.abs(out).mean())
